# revision 21
# baseline (speedup 1.0000x reference)
"""Multi-head attention kernel for Trainium2 (Bass/Tile), 8 NeuronCores.

Problem: nn_MultiHeadAttention  (B=4, S=2048, D=1024, H=16, DK=64)
    out = softmax((q Wq^T + bq)(k Wk^T + bk)^T / sqrt(DK)) (v Wv^T + bv) Wo^T + bo

Sharding: core c = 2*b + g handles batch b and head-group g (8 heads = 512
features).  Each core computes its batch's attention for its heads plus a
partial output projection; the host sums the two partials per batch.

Math simplifications done on the host (exact):
  - k-bias bk drops out (softmax shift invariance along keys).
  - v-bias bv folds into an effective output bias bo_eff = bo + Wo @ bv.
  - the 1/sqrt(DK) logit scale is folded into Wq/bq.

v2 design (PE-quadrant tiling + pair pipeline):
  - Heads processed in even/odd PAIRS.  S^T matmuls have K=64 contraction,
    so the even head (features on partitions 0:64) and odd head (64:128)
    auto-derive tile_position (0,0)/(64,0) and run CONCURRENTLY on separate
    PE row-groups (~2x).  PV matmuls have M=64 outputs; even head writes
    PSUM partitions 0:64, odd head 64:128 -> col-tiled concurrent (~2x).
  - Softmax denominators come from M=1 ones-matmuls, col-tiled 4-up at
    PSUM partitions {0,32,64,96}, accumulated over key chunks like PV.
  - Queries are processed in two 1024-column halves per pair so PSUM fits:
    S^T slots 2x[128,1024] (4 banks) + PV accum 2x[128,512] (2) + den (1)
    + background-projection psum (1) = 8 banks.
  - ScalarE exp stream ([128,1024] tiles, ~250us total) is the target
    critical path; Q/K/V/output projections are issued as background PE
    tasks interleaved one per pipeline step under the exp stream.
  - Normalization: DVE reciprocal on the 4 denominator PSUM rows, GpSimd
    partition_broadcast (otherwise-idle engine) to spread 1/den across
    partitions, DVE multiply into O.
"""

import numpy as np
import ml_dtypes
from collections import deque
from contextlib import ExitStack

import concourse.bass as bass
import concourse.tile as tile
from concourse import bacc, mybir
from concourse.bass import ts, ds
from concourse.bass_utils import run_bass_kernel_spmd

B, S, D, H, DK = 4, 2048, 1024, 16, 64
N_CORES = 8
F32 = mybir.dt.float32
BF16 = mybir.dt.bfloat16
AF = mybir.ActivationFunctionType
ALU = mybir.AluOpType
BF16NP = ml_dtypes.bfloat16


def build_nc(s: int = S):
    """Build + compile the per-core Bass module (SPMD: same NEFF, per-core data)."""
    assert s % 1024 == 0
    nsi = s // 128   # 128-row key chunks
    nf = s // 512    # 512-col query chunks
    nhalf = s // 1024  # 1024-col query halves

    nc = bacc.Bacc("TRN2", target_bir_lowering=False, debug=False)

    qT = nc.dram_tensor("qT", [D, s], BF16, kind="ExternalInput").ap()
    kT = nc.dram_tensor("kT", [D, s], BF16, kind="ExternalInput").ap()
    vT = nc.dram_tensor("vT", [D, s], BF16, kind="ExternalInput").ap()
    wq = nc.dram_tensor("wq", [D, 512], BF16, kind="ExternalInput").ap()
    wk = nc.dram_tensor("wk", [D, 512], BF16, kind="ExternalInput").ap()
    wv = nc.dram_tensor("wv", [D, 512], BF16, kind="ExternalInput").ap()
    wo = nc.dram_tensor("wo", [512, D], BF16, kind="ExternalInput").ap()
    bq = nc.dram_tensor("bq", [128, 4], F32, kind="ExternalInput").ap()
    outT = nc.dram_tensor("outT", [D, s], F32, kind="ExternalOutput").ap()

    with tile.TileContext(nc) as tc, ExitStack() as ctx:
        pers = ctx.enter_context(tc.tile_pool(name="pers", bufs=1))
        pst = ctx.enter_context(tc.tile_pool(name="pst", bufs=2, space="PSUM"))
        ppv = ctx.enter_context(tc.tile_pool(name="ppv", bufs=2, space="PSUM"))
        pden = ctx.enter_context(tc.tile_pool(name="pden", bufs=1, space="PSUM"))
        pproj = ctx.enter_context(tc.tile_pool(name="pproj", bufs=1, space="PSUM"))
        xpool = ctx.enter_context(tc.tile_pool(name="x", bufs=24))
        epool = ctx.enter_context(tc.tile_pool(name="e", bufs=6))
        oupool = ctx.enter_context(tc.tile_pool(name="ou", bufs=4))
        bpool = ctx.enter_context(tc.tile_pool(name="b", bufs=4))
        rpool = ctx.enter_context(tc.tile_pool(name="r", bufs=2))
        dpool = ctx.enter_context(tc.tile_pool(name="dsc", bufs=8, space="DRAM"))
        opool = ctx.enter_context(tc.tile_pool(name="ot", bufs=3))

        QT = pers.tile([128, 4, s], BF16)      # Q'^T  [feat, seq]; feat=(h%2)*64+d, blk=pair
        KT = pers.tile([128, 4, s], BF16)      # K^T   same layout
        V = pers.tile([128, nsi, 8, 64], BF16)  # V natural [seq, si, head, dv]
        O = pers.tile([128, 4, s], BF16)       # normalized attention out, same layout as QT
        WO = pers.tile([128, 4, D], BF16)
        WQ = pers.tile([128, 8, 512], BF16)
        WK = pers.tile([128, 8, 512], BF16)
        WV = pers.tile([128, 8, 512], BF16)
        BQ = pers.tile([128, 4], F32)
        ONES = pers.tile([128, 1], BF16)

        nc.vector.memset(ONES[:], 1.0)

        outr = outT.rearrange("(o p) n -> p o n", p=128)

        # ---- background task machinery -------------------------------
        # Each task is a closure issuing ~1-2.5us of PE work (+DVE/DMA).
        bg = deque()
        psum_rot = [0]
        in_attention = [False]

        def bg_psum():
            """Background psum. Outside the attention phase, rotate between
            the pproj bank and a pst view so consecutive chunks don't
            serialize on one bank. During attention pst holds live S^T
            slots, so only pproj is available; pops are spaced apart by
            attention-critical PE work to hide the copy-out latency."""
            if in_attention[0]:
                return pproj.tile([128, 512], F32, tag="pp", name="bgp")
            psum_rot[0] ^= 1
            if psum_rot[0]:
                return pproj.tile([128, 512], F32, tag="pp", name="bgp")
            return pst.tile([128, 1024], F32, tag="st", name="bgst")[:, 0:512]

        def bg_pop(n=1):
            for _ in range(n):
                if bg:
                    bg.popleft()()

        def dma_x_pair(xdram, fp):
            """Stage one 1024-seq-col x block (two f-chunks) as 8 [128,1024]
            tiles -- 2KB/partition DMA rows for descriptor efficiency."""
            xts = []
            for ki in range(8):
                xt = xpool.tile([128, 1024], BF16, tag="x")
                nc.sync.dma_start(xt[:], xdram[ds(ki * 128, 128), ds(fp * 1024, 1024)])
                xts.append(xt)
            return xts

        def proj_chunk(wt, xts, pc, f, dst, bias):
            """One [128 feats, 512 seq] projection psum tile + copy-out."""
            fl = f % 2
            ps = bg_psum()
            for ki in range(8):
                nc.tensor.matmul(
                    ps[:],
                    lhsT=wt[:, ki, ts(pc, 128)],
                    rhs=xts[ki][:, ts(fl, 512)],
                    start=(ki == 0),
                    stop=(ki == 7),
                )
            if bias is not None:
                nc.vector.tensor_scalar_add(dst, ps[:], bias)
            else:
                nc.vector.tensor_copy(dst, ps[:])

        def vproj_sj(xts, f, sj, pg):
            """V' for seq rows f*512+sj*128, pair-group pg (pairs 2pg,2pg+1)."""
            si = f * 4 + sj
            fl = f % 2
            ps = bg_psum()
            for ki in range(8):
                nc.tensor.matmul(
                    ps[:, 0:256],
                    lhsT=xts[ki][:, ds(fl * 512 + sj * 128, 128)],
                    rhs=WV[:, ki, ds(pg * 256, 256)],
                    start=(ki == 0),
                    stop=(ki == 7),
                )
            nc.vector.tensor_copy(
                V[:, si, ds(pg * 4, 4), :],
                ps[:, 0:256].rearrange("p (h d) -> p h d", h=4),
            )

        def outproj_chunk(pe, f):
            ps = bg_psum()
            for ki in range(4):
                nc.tensor.matmul(
                    ps[:],
                    lhsT=WO[:, ki, ts(pe, 128)],
                    rhs=O[:, ki, ts(f, 512)],
                    start=(ki == 0),
                    stop=(ki == 3),
                )
            ot = opool.tile([128, 512], F32, tag="ot")
            nc.vector.tensor_copy(ot[:], ps[:])
            nc.sync.dma_start(outr[:, pe, ts(f, 512)], ot[:])

        # ---- attention pipeline --------------------------------------
        def qk_slot(pair, half, si, fo):
            """Row-tiled concurrent S^T pair for one 512-col query chunk.
            Slot layout: cols 0:512 even head, 512:1024 odd head."""
            st = pst.tile([128, 1024], F32, tag="st", name=f"st{pair}_{half}_{si}_{fo}")
            qof = half * 1024 + fo * 512
            for ho in range(2):
                hh = ho * 64
                nc.tensor.matmul(
                    st[:, ts(ho, 512)],
                    lhsT=KT[ds(hh, 64), pair, ts(si, 128)],
                    rhs=QT[ds(hh, 64), pair, ds(qof, 512)],
                    start=True,
                    stop=True,
                )
            e = epool.tile([128, 1024], BF16, tag="e", name=f"e{pair}_{half}_{si}_{fo}")
            nc.scalar.activation(e[:], st[:], AF.Exp)
            return e

        def pv_den(pair, si, es, pos, dps):
            for fo in range(2):
                e = es[fo]
                for ho in range(2):  # col-tiled concurrent PV pair
                    nc.tensor.matmul(
                        pos[fo][ds(ho * 64, 64), :],
                        lhsT=V[:, si, pair * 2 + ho, :],
                        rhs=e[:, ts(ho, 512)],
                        start=(si == 0),
                        stop=(si == nsi - 1),
                        skip_group_check=True,
                    )
            # denominator: 4-up col-tiled M=1 ones-matmuls
            for r, fo, ho in ((0, 0, 0), (32, 1, 0), (64, 0, 1), (96, 1, 1)):
                nc.tensor.matmul(
                    dps[ds(r, 1), :],
                    lhsT=ONES[:, :],
                    rhs=es[fo][:, ts(ho, 512)],
                    start=(si == 0),
                    stop=(si == nsi - 1),
                    tile_position=(0, r),
                    skip_group_check=True,
                )

        def finish_half(pair, half, pos, dps):
            """Copy PV accum out of PSUM, 1/den, broadcast, normalize into O."""
            ou = []
            for fo in range(2):
                t = oupool.tile([128, 512], F32, tag="ou", name=f"ou{pair}_{half}_{fo}")
                nc.vector.tensor_copy(t[:], pos[fo][:])
                ou.append(t)
            # Free the den bank fast (4 cheap copies), then compute 1/den
            # off the critical path: reshape the 4x512 denominators to
            # [64, 32] via a DRAM bounce so one short DVE reciprocal covers
            # them (reciprocal costs ~6.5ns/free-elem), bounce back, and
            # partition-broadcast via DMA.
            dcp = rpool.tile([128, 512], F32, tag="rr", name=f"dcp{pair}_{half}")
            for r in (0, 32, 64, 96):
                nc.vector.tensor_copy(dcp[ds(r, 1), :], dps[ds(r, 1), :])
            dd = dpool.tile([4, 512], F32, tag="dd", name=f"dd{pair}_{half}")
            nc.sync.dma_start(
                dd[:], dcp[:].rearrange("(a b) c -> a b c", a=4)[:, 0, :]
            )
            d64 = bpool.tile([64, 32], F32, tag="d64", name=f"d64{pair}_{half}")
            nc.sync.dma_start(d64[:], dd[:].rearrange("a (p c) -> (a p) c", p=16))
            r64 = bpool.tile([64, 32], F32, tag="r64", name=f"r64{pair}_{half}")
            nc.vector.reciprocal(r64[:], d64[:])
            dd2 = dpool.tile([4, 512], F32, tag="dd2", name=f"dd2{pair}_{half}")
            nc.sync.dma_start(dd2[:].rearrange("a (p c) -> (a p) c", p=16), r64[:])
            for fo in range(2):
                bsb = bpool.tile([128, 512], F32, tag="bsb", name=f"bsb{pair}_{half}_{fo}")
                nc.sync.dma_start(bsb[ds(0, 64), :], dd2[ds(fo, 1), :].to_broadcast((64, 512)))
                nc.sync.dma_start(bsb[ds(64, 64), :], dd2[ds(2 + fo, 1), :].to_broadcast((64, 512)))
                nc.vector.tensor_tensor(
                    O[:, pair, ds(half * 1024 + fo * 512, 512)],
                    ou[fo][:],
                    bsb[:],
                    ALU.mult,
                )

        def attention_half(pair, half):
            pos = [
                ppv.tile([128, 512], F32, tag="pos", name=f"pos{pair}_{half}_{fo}")
                for fo in range(2)
            ]
            dps = pden.tile([128, 512], F32, tag="den", name=f"den{pair}_{half}")
            prev = None
            for si in range(nsi):
                cur = [qk_slot(pair, half, si, 0), qk_slot(pair, half, si, 1)]
                bg_pop(1)
                if prev is not None:
                    pv_den(pair, si - 1, prev, pos, dps)
                prev = cur
                bg_pop(1)
            pv_den(pair, nsi - 1, prev, pos, dps)
            finish_half(pair, half, pos, dps)

        # ---- phase A: lead-in projections (pair-0 critical path) -----
        # Minimal path to the first exp: K'(f0,pc0), Q'(f0/f1,pc0).
        # DMA order puts the lead's own x/weights first.  x stages are
        # f-PAIRS: kx[fp] covers f-chunks 2fp and 2fp+1.
        kx = {0: dma_x_pair(kT, 0)}
        nc.sync.dma_start(WK[:], wk.rearrange("(o p) m -> p o m", p=128))
        nc.sync.dma_start(WQ[:], wq.rearrange("(o p) m -> p o m", p=128))
        nc.sync.dma_start(BQ[:], bq)
        qx = {0: dma_x_pair(qT, 0)}
        proj_chunk(WK, kx[0], 0, 0, KT[:, 0, ts(0, 512)], None)
        nc.sync.dma_start(WV[:], wv.rearrange("(o p) m -> p o m", p=128))
        vx = {0: dma_x_pair(vT, 0)}
        proj_chunk(WQ, qx[0], 0, 0, QT[:, 0, ts(0, 512)], BQ[:, 0:1])
        proj_chunk(WQ, qx[0], 0, 1, QT[:, 0, ts(1, 512)], BQ[:, 0:1])
        nc.sync.dma_start(WO[:], wo.rearrange("(o p) e -> p o e", p=128))

        # ---- phase B: enqueue background tasks -----------------------
        # Deadline-ordered: pair 0's V'/K' needs first (V sj before its PV,
        # K' f-chunk before its S^T si range), then Q' half-1 chunks, then
        # later pairs' projections (pc = pair, needed before that pair).
        # x stages are f-pairs; each stage's consumers follow within a few
        # pops so the 3-group xpool rotation stays safe.
        def stage_v(fp):
            vx[fp] = dma_x_pair(vT, fp)

        def stage_k(fp):
            kx[fp] = dma_x_pair(kT, fp)

        def stage_q(fp):
            qx[fp] = dma_x_pair(qT, fp)

        def add(fn, *a):
            bg.append(lambda: fn(*a))

        def add_v(f, sj, pg):
            bg.append(lambda: vproj_sj(vx[f // 2], f, sj, pg))

        def add_k(f, pc):
            bg.append(lambda: proj_chunk(WK, kx[f // 2], pc, f, KT[:, pc, ts(f, 512)], None))

        def add_q(f, pc):
            bg.append(
                lambda: proj_chunk(WQ, qx[f // 2], pc, f, QT[:, pc, ts(f, 512)], BQ[:, pc : pc + 1])
            )

        add_k(1, 0)
        add_v(0, 0, 0)
        add_v(0, 1, 0)
        add(stage_k, 1)
        add_v(0, 2, 0)
        add_v(0, 3, 0)
        add(stage_v, 1)
        add_k(2, 0)
        add_v(1, 0, 0)
        add_v(1, 1, 0)
        add_k(3, 0)
        add_v(1, 2, 0)
        add_v(1, 3, 0)
        add(stage_q, 1)
        add_v(2, 0, 0)
        add_v(2, 1, 0)
        add_q(2, 0)
        add_v(2, 2, 0)
        add_v(2, 3, 0)
        add_q(3, 0)
        add_v(3, 0, 0)
        add_v(3, 1, 0)
        add_v(3, 2, 0)
        add_v(3, 3, 0)

        # later pairs' K'/Q' (pc 1..3) + V' pair-group 1 -- re-stage x
        # per f-pair on demand, stages >=2 pops ahead of their consumers.
        for pc in range(1, 4):
            add(stage_k, 0)
            add_k(0, pc)
            add_k(1, pc)
            add(stage_k, 1)
            add_k(2, pc)
            add_k(3, pc)
            add(stage_q, 0)
            add_q(0, pc)
            add_q(1, pc)
            add(stage_q, 1)
            add_q(2, pc)
            add_q(3, pc)
            if pc == 1:
                # V' pair-group 1 (pairs 2,3) needed before pair 2
                add(stage_v, 0)
                for sj in range(4):
                    add_v(0, sj, 1)
                for sj in range(2):
                    add_v(1, sj, 1)
                add(stage_v, 1)
                for sj in range(2, 4):
                    add_v(1, sj, 1)
                for f in (2, 3):
                    for sj in range(4):
                        add_v(f, sj, 1)

        # ---- attention: 4 pairs x 2 query halves ---------------------
        in_attention[0] = True
        for pair in range(4):
            for half in range(nhalf):
                if pair == 3 and half == nhalf - 1:
                    # remaining bg should be empty by now; overlap some
                    # first-half output-projection chunks (which only
                    # need O columns 0:1024, complete after pair3/half0)
                    # under the last attention half.
                    for pe in range(4):
                        for f in range(2):
                            add(outproj_chunk, pe, f)
                attention_half(pair, half)
        in_attention[0] = False
        bg_pop(len(bg))

        # ---- tail: output projection ---------------------------------
        for pe in range(4, 8):
            for f in range(2):
                outproj_chunk(pe, f)
        for pe in range(8):
            for f in range(2, nf):
                outproj_chunk(pe, f)

    nc.compile()
    return nc


_NC_CACHE: dict = {}


def get_nc(s: int = S):
    if s not in _NC_CACHE:
        _NC_CACHE[s] = build_nc(s)
    return _NC_CACHE[s]


def _prep_in_maps(q, k, v, Wq, bq, Wk, Wv, Wo):
    """Host-side shard prep: per-core input dicts (cheap numpy reshapes)."""
    f32 = np.float32
    scale = 1.0 / np.sqrt(DK)
    xT = {}
    for b in range(B):
        xT[b] = (
            np.ascontiguousarray(q[b].T).astype(BF16NP),
            np.ascontiguousarray(k[b].T).astype(BF16NP),
            np.ascontiguousarray(v[b].T).astype(BF16NP),
        )
    per_g = {}
    for g in range(2):
        F = slice(512 * g, 512 * g + 512)
        per_g[g] = dict(
            wq=np.ascontiguousarray(Wq[F].T * scale).astype(BF16NP),
            wk=np.ascontiguousarray(Wk[F].T).astype(BF16NP),
            wv=np.ascontiguousarray(Wv[F].T).astype(BF16NP),
            wo=np.ascontiguousarray(Wo[:, F].T).astype(BF16NP),
            bq=np.ascontiguousarray(
                (bq[F] * scale).reshape(4, 128).T, dtype=f32
            ),
        )
    in_maps = []
    for c in range(N_CORES):
        b, g = c // 2, c % 2
        qb, kb, vb = xT[b]
        in_maps.append(dict(qT=qb, kT=kb, vT=vb, **per_g[g]))
    return in_maps


def kernel(q, k, v, Wq, bq, Wk, bk, Wv, bv, Wo, bo):
    q, k, v = (np.asarray(x, np.float32) for x in (q, k, v))
    Wq, bq, Wk, bk = (np.asarray(x, np.float32) for x in (Wq, bq, Wk, bk))
    Wv, bv, Wo, bo = (np.asarray(x, np.float32) for x in (Wv, bv, Wo, bo))

    nc = get_nc(S)
    in_maps = _prep_in_maps(q, k, v, Wq, bq, Wk, Wv, Wo)
    res = run_bass_kernel_spmd(nc, in_maps, core_ids=list(range(N_CORES)))

    # bk drops out of softmax; bv folds into an effective output bias.
    bo_eff = (
        bo.astype(np.float64) + Wo.astype(np.float64) @ bv.astype(np.float64)
    ).astype(np.float32)
    out = np.empty((B, S, D), np.float32)
    for b in range(B):
        acc = res.results[2 * b]["outT"] + res.results[2 * b + 1]["outT"]
        out[b] = acc.T + bo_eff
    return out


# revision 22
# speedup vs baseline: 1.1370x; 1.1370x over previous
"""Multi-head attention kernel for Trainium2 (Bass/Tile), 8 NeuronCores.

Problem: nn_MultiHeadAttention  (B=4, S=2048, D=1024, H=16, DK=64)
    out = softmax((q Wq^T + bq)(k Wk^T + bk)^T / sqrt(DK)) (v Wv^T + bv) Wo^T + bo

Sharding: core c = 2*b + g handles batch b and head-group g (8 heads = 512
features).  Each core computes its batch's attention for its heads plus a
partial output projection; the host sums the two partials per batch.

Math simplifications done on the host (exact):
  - k-bias bk drops out (softmax shift invariance along keys).
  - v-bias bv folds into an effective output bias bo_eff = bo + Wo @ bv.
  - the 1/sqrt(DK) logit scale is folded into Wq/bq.

v2 design (PE-quadrant tiling + pair pipeline):
  - Heads processed in even/odd PAIRS.  S^T matmuls have K=64 contraction,
    so the even head (features on partitions 0:64) and odd head (64:128)
    auto-derive tile_position (0,0)/(64,0) and run CONCURRENTLY on separate
    PE row-groups (~2x).  PV matmuls have M=64 outputs; even head writes
    PSUM partitions 0:64, odd head 64:128 -> col-tiled concurrent (~2x).
  - Softmax denominators come from M=1 ones-matmuls, col-tiled 4-up at
    PSUM partitions {0,32,64,96}, accumulated over key chunks like PV.
  - Queries are processed in two 1024-column halves per pair so PSUM fits:
    S^T slots 2x[128,1024] (4 banks) + PV accum 2x[128,512] (2) + den (1)
    + background-projection psum (1) = 8 banks.
  - ScalarE exp stream ([128,1024] tiles, ~250us total) is the target
    critical path; Q/K/V/output projections are issued as background PE
    tasks interleaved one per pipeline step under the exp stream.
  - Normalization: DVE reciprocal on the 4 denominator PSUM rows, GpSimd
    partition_broadcast (otherwise-idle engine) to spread 1/den across
    partitions, DVE multiply into O.
"""

import numpy as np
import ml_dtypes
from collections import deque
from contextlib import ExitStack

import concourse.bass as bass
import concourse.tile as tile
from concourse import bacc, mybir
from concourse.bass import ts, ds
from concourse.bass_utils import run_bass_kernel_spmd

B, S, D, H, DK = 4, 2048, 1024, 16, 64
N_CORES = 8
F32 = mybir.dt.float32
BF16 = mybir.dt.bfloat16
AF = mybir.ActivationFunctionType
ALU = mybir.AluOpType
BF16NP = ml_dtypes.bfloat16


def build_nc(s: int = S):
    """Build + compile the per-core Bass module (SPMD: same NEFF, per-core data)."""
    assert s % 1024 == 0
    nsi = s // 128   # 128-row key chunks
    nf = s // 512    # 512-col query chunks
    nhalf = s // 1024  # 1024-col query halves

    nc = bacc.Bacc("TRN2", target_bir_lowering=False, debug=False)

    qT = nc.dram_tensor("qT", [D, s], BF16, kind="ExternalInput").ap()
    kT = nc.dram_tensor("kT", [D, s], BF16, kind="ExternalInput").ap()
    vT = nc.dram_tensor("vT", [D, s], BF16, kind="ExternalInput").ap()
    wq = nc.dram_tensor("wq", [D, 512], BF16, kind="ExternalInput").ap()
    wk = nc.dram_tensor("wk", [D, 512], BF16, kind="ExternalInput").ap()
    wv = nc.dram_tensor("wv", [D, 512], BF16, kind="ExternalInput").ap()
    wo = nc.dram_tensor("wo", [512, D], BF16, kind="ExternalInput").ap()
    bq = nc.dram_tensor("bq", [128, 4], F32, kind="ExternalInput").ap()
    outT = nc.dram_tensor("outT", [D, s], F32, kind="ExternalOutput").ap()

    with tile.TileContext(nc) as tc, ExitStack() as ctx:
        pers = ctx.enter_context(tc.tile_pool(name="pers", bufs=1))
        pst = ctx.enter_context(tc.tile_pool(name="pst", bufs=2, space="PSUM"))
        ppv = ctx.enter_context(tc.tile_pool(name="ppv", bufs=2, space="PSUM"))
        pden = ctx.enter_context(tc.tile_pool(name="pden", bufs=1, space="PSUM"))
        pproj = ctx.enter_context(tc.tile_pool(name="pproj", bufs=1, space="PSUM"))
        xpool = ctx.enter_context(tc.tile_pool(name="x", bufs=3))
        epool = ctx.enter_context(tc.tile_pool(name="e", bufs=6))
        oupool = ctx.enter_context(tc.tile_pool(name="ou", bufs=4))
        bpool = ctx.enter_context(tc.tile_pool(name="b", bufs=4))
        rpool = ctx.enter_context(tc.tile_pool(name="r", bufs=2))
        dpool = ctx.enter_context(tc.tile_pool(name="dsc", bufs=8, space="DRAM"))
        opool = ctx.enter_context(tc.tile_pool(name="ot", bufs=3))

        QT = pers.tile([128, 4, s], BF16)      # Q'^T  [feat, seq]; feat=(h%2)*64+d, blk=pair
        KT = pers.tile([128, 4, s], BF16)      # K^T   same layout
        V = pers.tile([128, nsi, 8, 64], BF16)  # V natural [seq, si, head, dv]
        O = pers.tile([128, 4, s], BF16)       # normalized attention out, same layout as QT
        WO = pers.tile([128, 4, D], BF16)
        WQ = pers.tile([128, 8, 512], BF16)
        WK = pers.tile([128, 8, 512], BF16)
        WV = pers.tile([128, 8, 512], BF16)
        BQ = pers.tile([128, 4], F32)
        ONES = pers.tile([128, 1], BF16)

        nc.vector.memset(ONES[:], 1.0)

        outr = outT.rearrange("(o p) n -> p o n", p=128)

        # ---- background task machinery -------------------------------
        # Each task is a closure issuing ~1-2.5us of PE work (+DVE/DMA).
        bg = deque()
        psum_rot = [0]
        in_attention = [False]

        def bg_psum():
            """Background psum. Outside the attention phase, rotate between
            the pproj bank and a pst view so consecutive chunks don't
            serialize on one bank. During attention pst holds live S^T
            slots, so only pproj is available; pops are spaced apart by
            attention-critical PE work to hide the copy-out latency."""
            if in_attention[0]:
                return pproj.tile([128, 512], F32, tag="pp", name="bgp")
            psum_rot[0] ^= 1
            if psum_rot[0]:
                return pproj.tile([128, 512], F32, tag="pp", name="bgp")
            return pst.tile([128, 1024], F32, tag="st", name="bgst")[:, 0:512]

        def bg_pop(n=1):
            for _ in range(n):
                if bg:
                    bg.popleft()()

        def dma_x_pair(xdram, fp):
            """Stage one 1024-seq-col x block (two f-chunks, all 8 ki rows)
            with a SINGLE dma_start (SP sequencer setup is ~565ns each) and
            2KB/partition descriptor rows."""
            xt = xpool.tile([128, 8, 1024], BF16, tag="x")
            nc.sync.dma_start(
                xt[:], xdram.rearrange("(o p) c -> p o c", p=128)[:, :, ds(fp * 1024, 1024)]
            )
            return xt

        def proj_chunk(wt, xts, pc, f, dst, bias):
            """One [128 feats, 512 seq] projection psum tile + copy-out."""
            fl = f % 2
            ps = bg_psum()
            for ki in range(8):
                nc.tensor.matmul(
                    ps[:],
                    lhsT=wt[:, ki, ts(pc, 128)],
                    rhs=xts[:, ki, ts(fl, 512)],
                    start=(ki == 0),
                    stop=(ki == 7),
                )
            if bias is not None:
                nc.vector.tensor_scalar_add(dst, ps[:], bias)
            else:
                nc.vector.tensor_copy(dst, ps[:])

        def vproj_sj(xts, f, sj, pg):
            """V' for seq rows f*512+sj*128, pair-group pg (pairs 2pg,2pg+1)."""
            si = f * 4 + sj
            fl = f % 2
            ps = bg_psum()
            for ki in range(8):
                nc.tensor.matmul(
                    ps[:, 0:256],
                    lhsT=xts[:, ki, ds(fl * 512 + sj * 128, 128)],
                    rhs=WV[:, ki, ds(pg * 256, 256)],
                    start=(ki == 0),
                    stop=(ki == 7),
                )
            nc.vector.tensor_copy(
                V[:, si, ds(pg * 4, 4), :],
                ps[:, 0:256].rearrange("p (h d) -> p h d", h=4),
            )

        def outproj_chunk(pe, f):
            ps = bg_psum()
            for ki in range(4):
                nc.tensor.matmul(
                    ps[:],
                    lhsT=WO[:, ki, ts(pe, 128)],
                    rhs=O[:, ki, ts(f, 512)],
                    start=(ki == 0),
                    stop=(ki == 3),
                )
            ot = opool.tile([128, 512], F32, tag="ot")
            nc.vector.tensor_copy(ot[:], ps[:])
            nc.sync.dma_start(outr[:, pe, ts(f, 512)], ot[:])

        # ---- attention pipeline --------------------------------------
        def qk_slot(pair, half, si, fo):
            """Row-tiled concurrent S^T pair for one 512-col query chunk.
            Slot layout: cols 0:512 even head, 512:1024 odd head."""
            st = pst.tile([128, 1024], F32, tag="st", name=f"st{pair}_{half}_{si}_{fo}")
            qof = half * 1024 + fo * 512
            for ho in range(2):
                hh = ho * 64
                nc.tensor.matmul(
                    st[:, ts(ho, 512)],
                    lhsT=KT[ds(hh, 64), pair, ts(si, 128)],
                    rhs=QT[ds(hh, 64), pair, ds(qof, 512)],
                    start=True,
                    stop=True,
                )
            e = epool.tile([128, 1024], BF16, tag="e", name=f"e{pair}_{half}_{si}_{fo}")
            nc.scalar.activation(e[:], st[:], AF.Exp)
            return e

        def pv_den(pair, si, es, pos, dps):
            for fo in range(2):
                e = es[fo]
                for ho in range(2):  # col-tiled concurrent PV pair
                    nc.tensor.matmul(
                        pos[fo][ds(ho * 64, 64), :],
                        lhsT=V[:, si, pair * 2 + ho, :],
                        rhs=e[:, ts(ho, 512)],
                        start=(si == 0),
                        stop=(si == nsi - 1),
                        skip_group_check=True,
                    )
            # denominator: 4-up col-tiled M=1 ones-matmuls
            for r, fo, ho in ((0, 0, 0), (32, 1, 0), (64, 0, 1), (96, 1, 1)):
                nc.tensor.matmul(
                    dps[ds(r, 1), :],
                    lhsT=ONES[:, :],
                    rhs=es[fo][:, ts(ho, 512)],
                    start=(si == 0),
                    stop=(si == nsi - 1),
                    tile_position=(0, r),
                    skip_group_check=True,
                )

        def finish_half(pair, half, pos, dps):
            """Copy PV accum out of PSUM, 1/den, broadcast, normalize into O."""
            ou = []
            for fo in range(2):
                t = oupool.tile([128, 512], F32, tag="ou", name=f"ou{pair}_{half}_{fo}")
                nc.vector.tensor_copy(t[:], pos[fo][:])
                ou.append(t)
            # Free the den bank fast (4 cheap copies), then compute 1/den
            # off the critical path: reshape the 4x512 denominators to
            # [64, 32] via a DRAM bounce so one short DVE reciprocal covers
            # them (reciprocal costs ~6.5ns/free-elem), bounce back, and
            # partition-broadcast via DMA.
            dcp = rpool.tile([128, 512], F32, tag="rr", name=f"dcp{pair}_{half}")
            for r in (0, 32, 64, 96):
                nc.vector.tensor_copy(dcp[ds(r, 1), :], dps[ds(r, 1), :])
            dd = dpool.tile([4, 512], F32, tag="dd", name=f"dd{pair}_{half}")
            nc.sync.dma_start(
                dd[:], dcp[:].rearrange("(a b) c -> a b c", a=4)[:, 0, :]
            )
            d64 = bpool.tile([64, 32], F32, tag="d64", name=f"d64{pair}_{half}")
            nc.sync.dma_start(d64[:], dd[:].rearrange("a (p c) -> (a p) c", p=16))
            r64 = bpool.tile([64, 32], F32, tag="r64", name=f"r64{pair}_{half}")
            nc.vector.reciprocal(r64[:], d64[:])
            dd2 = dpool.tile([4, 512], F32, tag="dd2", name=f"dd2{pair}_{half}")
            nc.sync.dma_start(dd2[:].rearrange("a (p c) -> (a p) c", p=16), r64[:])
            for fo in range(2):
                bsb = bpool.tile([128, 512], F32, tag="bsb", name=f"bsb{pair}_{half}_{fo}")
                nc.sync.dma_start(bsb[ds(0, 64), :], dd2[ds(fo, 1), :].to_broadcast((64, 512)))
                nc.sync.dma_start(bsb[ds(64, 64), :], dd2[ds(2 + fo, 1), :].to_broadcast((64, 512)))
                nc.vector.tensor_tensor(
                    O[:, pair, ds(half * 1024 + fo * 512, 512)],
                    ou[fo][:],
                    bsb[:],
                    ALU.mult,
                )

        def attention_half(pair, half):
            pos = [
                ppv.tile([128, 512], F32, tag="pos", name=f"pos{pair}_{half}_{fo}")
                for fo in range(2)
            ]
            dps = pden.tile([128, 512], F32, tag="den", name=f"den{pair}_{half}")
            prev = None
            for si in range(nsi):
                cur = [qk_slot(pair, half, si, 0), qk_slot(pair, half, si, 1)]
                bg_pop(1)
                if prev is not None:
                    pv_den(pair, si - 1, prev, pos, dps)
                prev = cur
                bg_pop(1)
            pv_den(pair, nsi - 1, prev, pos, dps)
            finish_half(pair, half, pos, dps)

        # ---- phase A: lead-in projections (pair-0 critical path) -----
        # Minimal path to the first exp: K'(f0,pc0), Q'(f0/f1,pc0).
        # DMA order puts the lead's own x/weights first.  x stages are
        # f-PAIRS: kx[fp] covers f-chunks 2fp and 2fp+1.
        kx = {0: dma_x_pair(kT, 0)}
        nc.sync.dma_start(WK[:], wk.rearrange("(o p) m -> p o m", p=128))
        nc.sync.dma_start(WQ[:], wq.rearrange("(o p) m -> p o m", p=128))
        nc.sync.dma_start(BQ[:], bq)
        qx = {0: dma_x_pair(qT, 0)}
        proj_chunk(WK, kx[0], 0, 0, KT[:, 0, ts(0, 512)], None)
        nc.sync.dma_start(WV[:], wv.rearrange("(o p) m -> p o m", p=128))
        vx = {0: dma_x_pair(vT, 0)}
        proj_chunk(WQ, qx[0], 0, 0, QT[:, 0, ts(0, 512)], BQ[:, 0:1])
        proj_chunk(WQ, qx[0], 0, 1, QT[:, 0, ts(1, 512)], BQ[:, 0:1])
        nc.sync.dma_start(WO[:], wo.rearrange("(o p) e -> p o e", p=128))

        # ---- phase B: enqueue background tasks -----------------------
        # Deadline-ordered: pair 0's V'/K' needs first (V sj before its PV,
        # K' f-chunk before its S^T si range), then Q' half-1 chunks, then
        # later pairs' projections (pc = pair, needed before that pair).
        # x stages are f-pairs; each stage's consumers follow within a few
        # pops so the 3-group xpool rotation stays safe.
        def stage_v(fp):
            vx[fp] = dma_x_pair(vT, fp)

        def stage_k(fp):
            kx[fp] = dma_x_pair(kT, fp)

        def stage_q(fp):
            qx[fp] = dma_x_pair(qT, fp)

        def add(fn, *a):
            bg.append(lambda: fn(*a))

        def add_v(f, sj, pg):
            bg.append(lambda: vproj_sj(vx[f // 2], f, sj, pg))

        def add_k(f, pc):
            bg.append(lambda: proj_chunk(WK, kx[f // 2], pc, f, KT[:, pc, ts(f, 512)], None))

        def add_q(f, pc):
            bg.append(
                lambda: proj_chunk(WQ, qx[f // 2], pc, f, QT[:, pc, ts(f, 512)], BQ[:, pc : pc + 1])
            )

        add_k(1, 0)
        add_v(0, 0, 0)
        add_v(0, 1, 0)
        add(stage_k, 1)
        add_v(0, 2, 0)
        add_v(0, 3, 0)
        add(stage_v, 1)
        add_k(2, 0)
        add_v(1, 0, 0)
        add_v(1, 1, 0)
        add_k(3, 0)
        add_v(1, 2, 0)
        add_v(1, 3, 0)
        add(stage_q, 1)
        add_v(2, 0, 0)
        add_v(2, 1, 0)
        add_q(2, 0)
        add_v(2, 2, 0)
        add_v(2, 3, 0)
        add_q(3, 0)
        add_v(3, 0, 0)
        add_v(3, 1, 0)
        add_v(3, 2, 0)
        add_v(3, 3, 0)

        # later pairs' K'/Q' (pc 1..3) + V' pair-group 1 -- re-stage x
        # per f-pair on demand, stages >=2 pops ahead of their consumers.
        for pc in range(1, 4):
            add(stage_k, 0)
            add_k(0, pc)
            add_k(1, pc)
            add(stage_k, 1)
            add_k(2, pc)
            add_k(3, pc)
            add(stage_q, 0)
            add_q(0, pc)
            add_q(1, pc)
            add(stage_q, 1)
            add_q(2, pc)
            add_q(3, pc)
            if pc == 1:
                # V' pair-group 1 (pairs 2,3) needed before pair 2
                add(stage_v, 0)
                for sj in range(4):
                    add_v(0, sj, 1)
                for sj in range(2):
                    add_v(1, sj, 1)
                add(stage_v, 1)
                for sj in range(2, 4):
                    add_v(1, sj, 1)
                for f in (2, 3):
                    for sj in range(4):
                        add_v(f, sj, 1)

        # ---- attention: 4 pairs x 2 query halves ---------------------
        in_attention[0] = True
        for pair in range(4):
            for half in range(nhalf):
                if pair == 3 and half == nhalf - 1:
                    # remaining bg should be empty by now; overlap some
                    # first-half output-projection chunks (which only
                    # need O columns 0:1024, complete after pair3/half0)
                    # under the last attention half.
                    for pe in range(4):
                        for f in range(2):
                            add(outproj_chunk, pe, f)
                attention_half(pair, half)
        in_attention[0] = False
        bg_pop(len(bg))

        # ---- tail: output projection ---------------------------------
        for pe in range(4, 8):
            for f in range(2):
                outproj_chunk(pe, f)
        for pe in range(8):
            for f in range(2, nf):
                outproj_chunk(pe, f)

    nc.compile()
    return nc


_NC_CACHE: dict = {}


def get_nc(s: int = S):
    if s not in _NC_CACHE:
        _NC_CACHE[s] = build_nc(s)
    return _NC_CACHE[s]


def _prep_in_maps(q, k, v, Wq, bq, Wk, Wv, Wo):
    """Host-side shard prep: per-core input dicts (cheap numpy reshapes)."""
    f32 = np.float32
    scale = 1.0 / np.sqrt(DK)
    xT = {}
    for b in range(B):
        xT[b] = (
            np.ascontiguousarray(q[b].T).astype(BF16NP),
            np.ascontiguousarray(k[b].T).astype(BF16NP),
            np.ascontiguousarray(v[b].T).astype(BF16NP),
        )
    per_g = {}
    for g in range(2):
        F = slice(512 * g, 512 * g + 512)
        per_g[g] = dict(
            wq=np.ascontiguousarray(Wq[F].T * scale).astype(BF16NP),
            wk=np.ascontiguousarray(Wk[F].T).astype(BF16NP),
            wv=np.ascontiguousarray(Wv[F].T).astype(BF16NP),
            wo=np.ascontiguousarray(Wo[:, F].T).astype(BF16NP),
            bq=np.ascontiguousarray(
                (bq[F] * scale).reshape(4, 128).T, dtype=f32
            ),
        )
    in_maps = []
    for c in range(N_CORES):
        b, g = c // 2, c % 2
        qb, kb, vb = xT[b]
        in_maps.append(dict(qT=qb, kT=kb, vT=vb, **per_g[g]))
    return in_maps


def kernel(q, k, v, Wq, bq, Wk, bk, Wv, bv, Wo, bo):
    q, k, v = (np.asarray(x, np.float32) for x in (q, k, v))
    Wq, bq, Wk, bk = (np.asarray(x, np.float32) for x in (Wq, bq, Wk, bk))
    Wv, bv, Wo, bo = (np.asarray(x, np.float32) for x in (Wv, bv, Wo, bo))

    nc = get_nc(S)
    in_maps = _prep_in_maps(q, k, v, Wq, bq, Wk, Wv, Wo)
    res = run_bass_kernel_spmd(nc, in_maps, core_ids=list(range(N_CORES)))

    # bk drops out of softmax; bv folds into an effective output bias.
    bo_eff = (
        bo.astype(np.float64) + Wo.astype(np.float64) @ bv.astype(np.float64)
    ).astype(np.float32)
    out = np.empty((B, S, D), np.float32)
    for b in range(B):
        acc = res.results[2 * b]["outT"] + res.results[2 * b + 1]["outT"]
        out[b] = acc.T + bo_eff
    return out


# revision 23
# speedup vs baseline: 1.1879x; 1.0448x over previous
"""Multi-head attention kernel for Trainium2 (Bass/Tile), 8 NeuronCores.

Problem: nn_MultiHeadAttention  (B=4, S=2048, D=1024, H=16, DK=64)
    out = softmax((q Wq^T + bq)(k Wk^T + bk)^T / sqrt(DK)) (v Wv^T + bv) Wo^T + bo

Sharding: core c = 2*b + g handles batch b and head-group g (8 heads = 512
features).  Each core computes its batch's attention for its heads plus a
partial output projection; the host sums the two partials per batch.

Math simplifications done on the host (exact):
  - k-bias bk drops out (softmax shift invariance along keys).
  - v-bias bv folds into an effective output bias bo_eff = bo + Wo @ bv.
  - the 1/sqrt(DK) logit scale is folded into Wq/bq.

v2 design (PE-quadrant tiling + pair pipeline):
  - Heads processed in even/odd PAIRS.  S^T matmuls have K=64 contraction,
    so the even head (features on partitions 0:64) and odd head (64:128)
    auto-derive tile_position (0,0)/(64,0) and run CONCURRENTLY on separate
    PE row-groups (~2x).  PV matmuls have M=64 outputs; even head writes
    PSUM partitions 0:64, odd head 64:128 -> col-tiled concurrent (~2x).
  - Softmax denominators come from M=1 ones-matmuls, col-tiled 4-up at
    PSUM partitions {0,32,64,96}, accumulated over key chunks like PV.
  - Queries are processed in two 1024-column halves per pair so PSUM fits:
    S^T slots 2x[128,1024] (4 banks) + PV accum 2x[128,512] (2) + den (1)
    + background-projection psum (1) = 8 banks.
  - ScalarE exp stream ([128,1024] tiles, ~250us total) is the target
    critical path; Q/K/V/output projections are issued as background PE
    tasks interleaved one per pipeline step under the exp stream.
  - Normalization: DVE reciprocal on the 4 denominator PSUM rows, GpSimd
    partition_broadcast (otherwise-idle engine) to spread 1/den across
    partitions, DVE multiply into O.
"""

import numpy as np
import ml_dtypes
from collections import deque
from contextlib import ExitStack

import concourse.bass as bass
import concourse.tile as tile
from concourse import bacc, mybir
from concourse.bass import ts, ds
from concourse.bass_utils import run_bass_kernel_spmd

B, S, D, H, DK = 4, 2048, 1024, 16, 64
N_CORES = 8
F32 = mybir.dt.float32
BF16 = mybir.dt.bfloat16
AF = mybir.ActivationFunctionType
ALU = mybir.AluOpType
BF16NP = ml_dtypes.bfloat16


def build_nc(s: int = S):
    """Build + compile the per-core Bass module (SPMD: same NEFF, per-core data)."""
    assert s % 1024 == 0
    nsi = s // 128   # 128-row key chunks
    nf = s // 512    # 512-col query chunks
    nhalf = s // 1024  # 1024-col query halves

    nc = bacc.Bacc("TRN2", target_bir_lowering=False, debug=False)

    qT = nc.dram_tensor("qT", [D, s], BF16, kind="ExternalInput").ap()
    kT = nc.dram_tensor("kT", [D, s], BF16, kind="ExternalInput").ap()
    vT = nc.dram_tensor("vT", [D, s], BF16, kind="ExternalInput").ap()
    wq = nc.dram_tensor("wq", [D, 512], BF16, kind="ExternalInput").ap()
    wk = nc.dram_tensor("wk", [D, 512], BF16, kind="ExternalInput").ap()
    wv = nc.dram_tensor("wv", [D, 512], BF16, kind="ExternalInput").ap()
    wo = nc.dram_tensor("wo", [512, D], BF16, kind="ExternalInput").ap()
    bq = nc.dram_tensor("bq", [128, 4], F32, kind="ExternalInput").ap()
    outT = nc.dram_tensor("outT", [D, s], BF16, kind="ExternalOutput").ap()

    with tile.TileContext(nc) as tc, ExitStack() as ctx:
        pers = ctx.enter_context(tc.tile_pool(name="pers", bufs=1))
        pst = ctx.enter_context(tc.tile_pool(name="pst", bufs=2, space="PSUM"))
        ppv = ctx.enter_context(tc.tile_pool(name="ppv", bufs=2, space="PSUM"))
        pden = ctx.enter_context(tc.tile_pool(name="pden", bufs=1, space="PSUM"))
        pproj = ctx.enter_context(tc.tile_pool(name="pproj", bufs=1, space="PSUM"))
        xpool = ctx.enter_context(tc.tile_pool(name="x", bufs=3))
        epool = ctx.enter_context(tc.tile_pool(name="e", bufs=6))
        oupool = ctx.enter_context(tc.tile_pool(name="ou", bufs=4))
        bpool = ctx.enter_context(tc.tile_pool(name="b", bufs=4))
        rpool = ctx.enter_context(tc.tile_pool(name="r", bufs=2))
        dpool = ctx.enter_context(tc.tile_pool(name="dsc", bufs=8, space="DRAM"))
        opool = ctx.enter_context(tc.tile_pool(name="ot", bufs=3))

        QT = pers.tile([128, 4, s], BF16)      # Q'^T  [feat, seq]; feat=(h%2)*64+d, blk=pair
        KT = pers.tile([128, 4, s], BF16)      # K^T   same layout
        V = pers.tile([128, nsi, 8, 64], BF16)  # V natural [seq, si, head, dv]
        O = pers.tile([128, 4, s], BF16)       # normalized attention out, same layout as QT
        WO = pers.tile([128, 4, D], BF16)
        WQ = pers.tile([128, 8, 512], BF16)
        WK = pers.tile([128, 8, 512], BF16)
        WV = pers.tile([128, 8, 512], BF16)
        BQ = pers.tile([128, 4], F32)
        ONES = pers.tile([128, 1], BF16)

        nc.vector.memset(ONES[:], 1.0)

        outr = outT.rearrange("(o p) n -> p o n", p=128)

        # ---- background task machinery -------------------------------
        # Each task is a closure issuing ~1-2.5us of PE work (+DVE/DMA).
        bg = deque()
        psum_rot = [0]
        in_attention = [False]

        def bg_psum():
            """Background psum. Outside the attention phase, rotate between
            the pproj bank and a pst view so consecutive chunks don't
            serialize on one bank. During attention pst holds live S^T
            slots, so only pproj is available; pops are spaced apart by
            attention-critical PE work to hide the copy-out latency."""
            if in_attention[0]:
                return pproj.tile([128, 512], F32, tag="pp", name="bgp")
            psum_rot[0] ^= 1
            if psum_rot[0]:
                return pproj.tile([128, 512], F32, tag="pp", name="bgp")
            return pst.tile([128, 1024], F32, tag="st", name="bgst")[:, 0:512]

        def bg_pop(n=1):
            for _ in range(n):
                if bg:
                    bg.popleft()()

        def dma_x_pair(xdram, fp):
            """Stage one 1024-seq-col x block (two f-chunks, all 8 ki rows)
            with a SINGLE dma_start (SP sequencer setup is ~565ns each) and
            2KB/partition descriptor rows."""
            xt = xpool.tile([128, 8, 1024], BF16, tag="x")
            nc.sync.dma_start(
                xt[:], xdram.rearrange("(o p) c -> p o c", p=128)[:, :, ds(fp * 1024, 1024)]
            )
            return xt

        def proj_chunk(wt, xts, pc, f, dst, bias):
            """One [128 feats, 512 seq] projection psum tile + copy-out."""
            fl = f % 2
            ps = bg_psum()
            for ki in range(8):
                nc.tensor.matmul(
                    ps[:],
                    lhsT=wt[:, ki, ts(pc, 128)],
                    rhs=xts[:, ki, ts(fl, 512)],
                    start=(ki == 0),
                    stop=(ki == 7),
                )
            if bias is not None:
                nc.vector.tensor_scalar_add(dst, ps[:], bias)
            else:
                nc.vector.tensor_copy(dst, ps[:])

        def vproj_sj(xts, f, sj, pg):
            """V' for seq rows f*512+sj*128, pair-group pg (pairs 2pg,2pg+1)."""
            si = f * 4 + sj
            fl = f % 2
            ps = bg_psum()
            for ki in range(8):
                nc.tensor.matmul(
                    ps[:, 0:256],
                    lhsT=xts[:, ki, ds(fl * 512 + sj * 128, 128)],
                    rhs=WV[:, ki, ds(pg * 256, 256)],
                    start=(ki == 0),
                    stop=(ki == 7),
                )
            nc.vector.tensor_copy(
                V[:, si, ds(pg * 4, 4), :],
                ps[:, 0:256].rearrange("p (h d) -> p h d", h=4),
            )

        def outproj_chunk(pe, fpair):
            """Two 512-col output chunks batched into one bf16 store DMA."""
            ot = opool.tile([128, 1024], BF16, tag="ot")
            for fl in range(2):
                ps = bg_psum()
                for ki in range(4):
                    nc.tensor.matmul(
                        ps[:],
                        lhsT=WO[:, ki, ts(pe, 128)],
                        rhs=O[:, ki, ds(fpair * 1024 + fl * 512, 512)],
                        start=(ki == 0),
                        stop=(ki == 3),
                    )
                nc.vector.tensor_copy(ot[:, ts(fl, 512)], ps[:])
            nc.sync.dma_start(outr[:, pe, ds(fpair * 1024, 1024)], ot[:])

        # ---- attention pipeline --------------------------------------
        def qk_slot(pair, half, si, fo):
            """Row-tiled concurrent S^T pair for one 512-col query chunk.
            Slot layout: cols 0:512 even head, 512:1024 odd head."""
            st = pst.tile([128, 1024], F32, tag="st", name=f"st{pair}_{half}_{si}_{fo}")
            qof = half * 1024 + fo * 512
            for ho in range(2):
                hh = ho * 64
                nc.tensor.matmul(
                    st[:, ts(ho, 512)],
                    lhsT=KT[ds(hh, 64), pair, ts(si, 128)],
                    rhs=QT[ds(hh, 64), pair, ds(qof, 512)],
                    start=True,
                    stop=True,
                )
            e = epool.tile([128, 1024], BF16, tag="e", name=f"e{pair}_{half}_{si}_{fo}")
            nc.scalar.activation(e[:], st[:], AF.Exp)
            return e

        def pv_den(pair, si, es, pos, dps):
            for fo in range(2):
                e = es[fo]
                for ho in range(2):  # col-tiled concurrent PV pair
                    nc.tensor.matmul(
                        pos[fo][ds(ho * 64, 64), :],
                        lhsT=V[:, si, pair * 2 + ho, :],
                        rhs=e[:, ts(ho, 512)],
                        start=(si == 0),
                        stop=(si == nsi - 1),
                        skip_group_check=True,
                    )
            # denominator: 4-up col-tiled M=1 ones-matmuls
            for r, fo, ho in ((0, 0, 0), (32, 1, 0), (64, 0, 1), (96, 1, 1)):
                nc.tensor.matmul(
                    dps[ds(r, 1), :],
                    lhsT=ONES[:, :],
                    rhs=es[fo][:, ts(ho, 512)],
                    start=(si == 0),
                    stop=(si == nsi - 1),
                    tile_position=(0, r),
                    skip_group_check=True,
                )

        def finish_half(pair, half, pos, dps):
            """Copy PV accum out of PSUM, 1/den, broadcast, normalize into O."""
            ou = []
            for fo in range(2):
                t = oupool.tile([128, 512], F32, tag="ou", name=f"ou{pair}_{half}_{fo}")
                nc.vector.tensor_copy(t[:], pos[fo][:])
                ou.append(t)
            # Free the den bank fast (4 cheap copies), then compute 1/den
            # off the critical path: reshape the 4x512 denominators to
            # [64, 32] via a DRAM bounce so one short DVE reciprocal covers
            # them (reciprocal costs ~6.5ns/free-elem), bounce back, and
            # partition-broadcast via DMA.
            dcp = rpool.tile([128, 512], F32, tag="rr", name=f"dcp{pair}_{half}")
            for r in (0, 32, 64, 96):
                nc.vector.tensor_copy(dcp[ds(r, 1), :], dps[ds(r, 1), :])
            dd = dpool.tile([4, 512], F32, tag="dd", name=f"dd{pair}_{half}")
            nc.sync.dma_start(
                dd[:], dcp[:].rearrange("(a b) c -> a b c", a=4)[:, 0, :]
            )
            d64 = bpool.tile([64, 32], F32, tag="d64", name=f"d64{pair}_{half}")
            nc.sync.dma_start(d64[:], dd[:].rearrange("a (p c) -> (a p) c", p=16))
            r64 = bpool.tile([64, 32], F32, tag="r64", name=f"r64{pair}_{half}")
            nc.vector.reciprocal(r64[:], d64[:])
            dd2 = dpool.tile([4, 512], F32, tag="dd2", name=f"dd2{pair}_{half}")
            nc.sync.dma_start(dd2[:].rearrange("a (p c) -> (a p) c", p=16), r64[:])
            for fo in range(2):
                bsb = bpool.tile([128, 512], F32, tag="bsb", name=f"bsb{pair}_{half}_{fo}")
                nc.sync.dma_start(bsb[ds(0, 64), :], dd2[ds(fo, 1), :].to_broadcast((64, 512)))
                nc.sync.dma_start(bsb[ds(64, 64), :], dd2[ds(2 + fo, 1), :].to_broadcast((64, 512)))
                nc.vector.tensor_tensor(
                    O[:, pair, ds(half * 1024 + fo * 512, 512)],
                    ou[fo][:],
                    bsb[:],
                    ALU.mult,
                )

        def attention_half(pair, half):
            pos = [
                ppv.tile([128, 512], F32, tag="pos", name=f"pos{pair}_{half}_{fo}")
                for fo in range(2)
            ]
            dps = pden.tile([128, 512], F32, tag="den", name=f"den{pair}_{half}")
            prev = None
            for si in range(nsi):
                cur = [qk_slot(pair, half, si, 0), qk_slot(pair, half, si, 1)]
                bg_pop(1)
                if prev is not None:
                    pv_den(pair, si - 1, prev, pos, dps)
                prev = cur
                bg_pop(1)
            pv_den(pair, nsi - 1, prev, pos, dps)
            finish_half(pair, half, pos, dps)

        # ---- phase A: lead-in projections (pair-0 critical path) -----
        # Minimal path to the first exp: K'(f0,pc0), Q'(f0/f1,pc0).
        # DMA order puts the lead's own x/weights first.  x stages are
        # f-PAIRS: kx[fp] covers f-chunks 2fp and 2fp+1.
        kx = {0: xpool.tile([128, 8, 1024], BF16, tag="x", name="kx0")}
        kre = kT.rearrange("(o p) c -> p o c", p=128)
        nc.sync.dma_start(kx[0][:, :, 0:512], kre[:, :, 0:512])
        nc.sync.dma_start(WK[:], wk.rearrange("(o p) m -> p o m", p=128))
        nc.sync.dma_start(kx[0][:, :, 512:1024], kre[:, :, 512:1024])
        nc.sync.dma_start(WQ[:], wq.rearrange("(o p) m -> p o m", p=128))
        nc.sync.dma_start(BQ[:], bq)
        qx = {0: xpool.tile([128, 8, 1024], BF16, tag="x", name="qx0")}
        qre = qT.rearrange("(o p) c -> p o c", p=128)
        nc.sync.dma_start(qx[0][:, :, 0:512], qre[:, :, 0:512])
        proj_chunk(WK, kx[0], 0, 0, KT[:, 0, ts(0, 512)], None)
        nc.sync.dma_start(qx[0][:, :, 512:1024], qre[:, :, 512:1024])
        nc.sync.dma_start(WV[:], wv.rearrange("(o p) m -> p o m", p=128))
        vx = {0: dma_x_pair(vT, 0)}
        proj_chunk(WQ, qx[0], 0, 0, QT[:, 0, ts(0, 512)], BQ[:, 0:1])
        proj_chunk(WQ, qx[0], 0, 1, QT[:, 0, ts(1, 512)], BQ[:, 0:1])
        nc.sync.dma_start(WO[:], wo.rearrange("(o p) e -> p o e", p=128))

        # ---- phase B: enqueue background tasks -----------------------
        # Deadline-ordered: pair 0's V'/K' needs first (V sj before its PV,
        # K' f-chunk before its S^T si range), then Q' half-1 chunks, then
        # later pairs' projections (pc = pair, needed before that pair).
        # x stages are f-pairs; each stage's consumers follow within a few
        # pops so the 3-group xpool rotation stays safe.
        def stage_v(fp):
            vx[fp] = dma_x_pair(vT, fp)

        def stage_k(fp):
            kx[fp] = dma_x_pair(kT, fp)

        def stage_q(fp):
            qx[fp] = dma_x_pair(qT, fp)

        def add(fn, *a):
            bg.append(lambda: fn(*a))

        def add_v(f, sj, pg):
            bg.append(lambda: vproj_sj(vx[f // 2], f, sj, pg))

        def add_k(f, pc):
            bg.append(lambda: proj_chunk(WK, kx[f // 2], pc, f, KT[:, pc, ts(f, 512)], None))

        def add_q(f, pc):
            bg.append(
                lambda: proj_chunk(WQ, qx[f // 2], pc, f, QT[:, pc, ts(f, 512)], BQ[:, pc : pc + 1])
            )

        add_k(1, 0)
        add_v(0, 0, 0)
        add_v(0, 1, 0)
        add(stage_k, 1)
        add_v(0, 2, 0)
        add_v(0, 3, 0)
        add(stage_v, 1)
        add_k(2, 0)
        add_v(1, 0, 0)
        add_v(1, 1, 0)
        add_k(3, 0)
        add_v(1, 2, 0)
        add_v(1, 3, 0)
        add(stage_q, 1)
        add_v(2, 0, 0)
        add_v(2, 1, 0)
        add_q(2, 0)
        add_v(2, 2, 0)
        add_v(2, 3, 0)
        add_q(3, 0)
        add_v(3, 0, 0)
        add_v(3, 1, 0)
        add_v(3, 2, 0)
        add_v(3, 3, 0)

        # later pairs' K'/Q' (pc 1..3) + V' pair-group 1 -- re-stage x
        # per f-pair on demand, stages >=2 pops ahead of their consumers.
        for pc in range(1, 4):
            add(stage_k, 0)
            add_k(0, pc)
            add_k(1, pc)
            add(stage_k, 1)
            add_k(2, pc)
            add_k(3, pc)
            add(stage_q, 0)
            add_q(0, pc)
            add_q(1, pc)
            add(stage_q, 1)
            add_q(2, pc)
            add_q(3, pc)
            if pc == 1:
                # V' pair-group 1 (pairs 2,3) needed before pair 2
                add(stage_v, 0)
                for sj in range(4):
                    add_v(0, sj, 1)
                for sj in range(2):
                    add_v(1, sj, 1)
                add(stage_v, 1)
                for sj in range(2, 4):
                    add_v(1, sj, 1)
                for f in (2, 3):
                    for sj in range(4):
                        add_v(f, sj, 1)

        # ---- attention: 4 pairs x 2 query halves ---------------------
        in_attention[0] = True
        for pair in range(4):
            for half in range(nhalf):
                if pair == 3 and half == nhalf - 1:
                    # remaining bg should be empty by now; overlap some
                    # first-half output-projection chunks (which only
                    # need O columns 0:1024, complete after pair3/half0)
                    # under the last attention half.
                    for pe in range(4):
                        add(outproj_chunk, pe, 0)
                attention_half(pair, half)
        in_attention[0] = False
        bg_pop(len(bg))

        # ---- tail: output projection ---------------------------------
        for pe in range(4, 8):
            outproj_chunk(pe, 0)
        for pe in range(8):
            outproj_chunk(pe, 1)

    nc.compile()
    return nc


_NC_CACHE: dict = {}


def get_nc(s: int = S):
    if s not in _NC_CACHE:
        _NC_CACHE[s] = build_nc(s)
    return _NC_CACHE[s]


def _prep_in_maps(q, k, v, Wq, bq, Wk, Wv, Wo):
    """Host-side shard prep: per-core input dicts (cheap numpy reshapes)."""
    f32 = np.float32
    scale = 1.0 / np.sqrt(DK)
    xT = {}
    for b in range(B):
        xT[b] = (
            np.ascontiguousarray(q[b].T).astype(BF16NP),
            np.ascontiguousarray(k[b].T).astype(BF16NP),
            np.ascontiguousarray(v[b].T).astype(BF16NP),
        )
    per_g = {}
    for g in range(2):
        F = slice(512 * g, 512 * g + 512)
        per_g[g] = dict(
            wq=np.ascontiguousarray(Wq[F].T * scale).astype(BF16NP),
            wk=np.ascontiguousarray(Wk[F].T).astype(BF16NP),
            wv=np.ascontiguousarray(Wv[F].T).astype(BF16NP),
            wo=np.ascontiguousarray(Wo[:, F].T).astype(BF16NP),
            bq=np.ascontiguousarray(
                (bq[F] * scale).reshape(4, 128).T, dtype=f32
            ),
        )
    in_maps = []
    for c in range(N_CORES):
        b, g = c // 2, c % 2
        qb, kb, vb = xT[b]
        in_maps.append(dict(qT=qb, kT=kb, vT=vb, **per_g[g]))
    return in_maps


def kernel(q, k, v, Wq, bq, Wk, bk, Wv, bv, Wo, bo):
    q, k, v = (np.asarray(x, np.float32) for x in (q, k, v))
    Wq, bq, Wk, bk = (np.asarray(x, np.float32) for x in (Wq, bq, Wk, bk))
    Wv, bv, Wo, bo = (np.asarray(x, np.float32) for x in (Wv, bv, Wo, bo))

    nc = get_nc(S)
    in_maps = _prep_in_maps(q, k, v, Wq, bq, Wk, Wv, Wo)
    res = run_bass_kernel_spmd(nc, in_maps, core_ids=list(range(N_CORES)))

    # bk drops out of softmax; bv folds into an effective output bias.
    bo_eff = (
        bo.astype(np.float64) + Wo.astype(np.float64) @ bv.astype(np.float64)
    ).astype(np.float32)
    out = np.empty((B, S, D), np.float32)
    for b in range(B):
        acc = res.results[2 * b]["outT"].astype(np.float32) + res.results[
            2 * b + 1
        ]["outT"].astype(np.float32)
        out[b] = acc.T + bo_eff
    return out
